# revision 60
# baseline (speedup 1.0000x reference)
"""Causal MHA with RoPE on 8 trn2 cores.

Sharding: core = (batch b, head-group g). b = core//2, g = core%2.
Each core computes 8 heads of one batch and a partial output projection;
host sums the two partials per batch.

Per-core pipeline:
  - Q/K/V projections as fp8(e4m3) hi/lo DoubleRow matmuls (3-term, host
    quantized with gamma=64 pre-scale; dequant folded into the psum->sbuf
    scaled copy on DVE).
  - RoPE on DVE (bf16), transposes via single 3D xbar DMAs.
  - scores [keys x q] bf16 with ragged causal extents, exp on ACT,
    post-exp 0/1 tri-mask on DVE for diagonal tiles, PV oriented
    [q x (64+1)] with causal tile skipping; softmax normalize via DVE
    reciprocal + broadcast mul.
  - output projection bf16; proj chunks and the previous block's output
    projection are spread through the attention pair stream as fillers.
"""

import numpy as np
import ml_dtypes

import concourse.bass as bass
from concourse import bacc
import concourse.mybir as mybir
import concourse.tile as tile
from concourse.bass_utils import run_bass_kernel_spmd

F32 = mybir.dt.float32
BF16 = mybir.dt.bfloat16
F8 = mybir.dt.float8e4
AF = mybir.ActivationFunctionType
DR = mybir.MatmulPerfMode.DoubleRow

P = 128
S = 2048          # sequence length
DM = 1024         # model dim
DH = 512          # per-core heads dim (8 heads x 64)
DK = 64
NHL = 8           # local heads
KT = DM // P      # 8 k-tiles for projections
SC = S // P       # 16 s-chunks
QB = S // 512     # 4 q-blocks
THETA = 10000.0
GAMMA = 64.0      # fp8 weight pre-scale


def _bcast_last(ap3, n):
    # [P, A, 1] -> [P, A, n] with stride-0 last dim
    assert len(ap3.ap) == 3 and ap3.ap[2][1] == 1
    return bass.AP(tensor=ap3.tensor, offset=ap3.offset,
                   ap=[ap3.ap[0], ap3.ap[1], [0, n]])


def _bcast_mid(ap2d, n):
    # [P, K] -> [P, n, K] with a stride-0 middle dim
    assert len(ap2d.ap) == 2
    return bass.AP(tensor=ap2d.tensor, offset=ap2d.offset,
                   ap=[ap2d.ap[0], [0, n], ap2d.ap[1]])


def build_program(repeat=1):
    nc = bacc.Bacc("TRN2", target_bir_lowering=False, debug=False)
    xh_d = nc.dram_tensor("xh", [P, KT, S], F8, kind="ExternalInput").ap()
    xl_d = nc.dram_tensor("xl", [P, KT, S], F8, kind="ExternalInput").ap()
    wqh_d = nc.dram_tensor("wqh", [P, KT, DH], F8, kind="ExternalInput").ap()
    wql_d = nc.dram_tensor("wql", [P, KT, DH], F8, kind="ExternalInput").ap()
    wkh_d = nc.dram_tensor("wkh", [P, KT, DH], F8, kind="ExternalInput").ap()
    wkl_d = nc.dram_tensor("wkl", [P, KT, DH], F8, kind="ExternalInput").ap()
    wvh_d = nc.dram_tensor("wvh", [P, KT, DH], F8, kind="ExternalInput").ap()
    wvl_d = nc.dram_tensor("wvl", [P, KT, DH], F8, kind="ExternalInput").ap()
    wot_d = nc.dram_tensor("wot", [P, 4, DM], BF16, kind="ExternalInput").ap()
    ct_d = nc.dram_tensor("ct", [P, SC, 32], BF16, kind="ExternalInput").ap()
    st_d = nc.dram_tensor("st", [P, SC, 64], BF16, kind="ExternalInput").ap()
    ntri_d = nc.dram_tensor("ntri", [P, P], BF16, kind="ExternalInput").ap()
    yt_d = nc.dram_tensor("yt", [DM, S], F32, kind="ExternalOutput").ap()

    with tile.TileContext(nc) as tc:
        with (
            tc.tile_pool(name="consts", bufs=1) as consts,
            tc.tile_pool(name="raw", bufs=6) as raw,
            tc.tile_pool(name="rop", bufs=8) as rop,
            tc.tile_pool(name="ptp", bufs=6) as ptp,
            tc.tile_pool(name="otn", bufs=6) as otn_pool,
            tc.tile_pool(name="ott", bufs=4) as ott_pool,
            tc.tile_pool(name="rlp", bufs=6) as rlp,
            tc.tile_pool(name="ysb", bufs=4) as ysb_pool,
            tc.tile_pool(name="yac", bufs=8) as yac_pool,
            tc.tile_pool(name="pssc", bufs=2, space="PSUM") as ps_sc,
            tc.tile_pool(name="pspv", bufs=2, space="PSUM") as ps_pv,
            tc.tile_pool(name="pspp", bufs=2, space="PSUM") as ps_pp,
        ):
            # ---- constants (ordered so the first proj chunk starts early) ----
            xh = consts.tile([P, KT, S], F8)
            xl = consts.tile([P, KT, S], F8)
            wqh = consts.tile([P, KT, DH], F8)
            nc.sync.dma_start(out=wqh, in_=wqh_d)
            nc.sync.dma_start(out=xh[:, :, 0:512], in_=xh_d[:, :, 0:512])
            wql = consts.tile([P, KT, DH], F8)
            nc.sync.dma_start(out=wql, in_=wql_d)
            nc.sync.dma_start(out=xl[:, :, 0:512], in_=xl_d[:, :, 0:512])
            wkh = consts.tile([P, KT, DH], F8)
            nc.sync.dma_start(out=wkh, in_=wkh_d)
            wkl = consts.tile([P, KT, DH], F8)
            nc.sync.dma_start(out=wkl, in_=wkl_d)
            wvh = consts.tile([P, KT, DH], F8)
            nc.sync.dma_start(out=wvh, in_=wvh_d)
            wvl = consts.tile([P, KT, DH], F8)
            nc.sync.dma_start(out=wvl, in_=wvl_d)
            ctbl = consts.tile([P, SC, 32], BF16)
            nc.sync.dma_start(out=ctbl, in_=ct_d)
            stbl = consts.tile([P, SC, 64], BF16)
            nc.sync.dma_start(out=stbl, in_=st_d)
            for sb in range(1, 4):
                nc.sync.dma_start(out=xh[:, :, sb * 512:(sb + 1) * 512],
                                  in_=xh_d[:, :, sb * 512:(sb + 1) * 512])
                nc.sync.dma_start(out=xl[:, :, sb * 512:(sb + 1) * 512],
                                  in_=xl_d[:, :, sb * 512:(sb + 1) * 512])
            ntri = consts.tile([P, P], BF16)
            nc.sync.dma_start(out=ntri, in_=ntri_d)
            wot = consts.tile([P, 4, DM], BF16)
            nc.sync.dma_start(out=wot, in_=wot_d)

            qt_store = consts.tile([P, 4, S], BF16)
            kt_store = consts.tile([P, 4, S], BF16)
            v_aug = consts.tile([P, SC, NHL, 65], BF16)
            nc.vector.memset(v_aug[:, :, :, 64:65], 1.0)

            # pre-zero the two score psum slots: ragged diagonal scores leave
            # the causally-dead prefix unwritten, but exp still reads it
            for _zi in range(2):
                zs = ps_sc.tile([P, 2, 512], F32, tag="sc")
                nc.vector.memset(zs, 0.0)

            # ---- projections + rope, per s-chunk ----
            def proj_mm(ps, xhi, xlo, whi, wlo, sl):
                # 3-term fp8 hi/lo DoubleRow: 12 matmuls, 256-contraction each
                terms = [(xhi, whi), (xhi, wlo), (xlo, whi)]
                for ti, (xa, wa) in enumerate(terms):
                    for t in range(4):
                        nc.tensor.matmul(
                            ps, xa[:, 2 * t:2 * t + 2, sl], wa[:, 2 * t:2 * t + 2, :],
                            start=(ti == 0 and t == 0),
                            stop=(ti == 2 and t == 3),
                            perf_mode=DR)

            def rope_and_transpose(pj, sc, dst_store, deq):
                qraw = raw.tile([P, DH], BF16, tag="qraw")
                nc.vector.tensor_scalar_mul(qraw, pj, deq)
                t1 = rop.tile([P, DH], BF16, tag="t1")
                t2 = rop.tile([P, DH], BF16, tag="t2")
                qv = qraw.rearrange("p (h two k) -> p h two k", two=2, k=32)
                t1v = t1.rearrange("p (h two k) -> p h two k", two=2, k=32)
                # evens-out: t1[:, h, 0, :] = qraw[:, h, 1, :] * (-sin)
                nc.vector.tensor_mul(
                    t1v[:, :, 0, :], qv[:, :, 1, :],
                    _bcast_mid(stbl[:, sc, 0:32], NHL))
                # odds-out: t1[:, h, 1, :] = qraw[:, h, 0, :] * (+sin)
                nc.vector.tensor_mul(
                    t1v[:, :, 1, :], qv[:, :, 0, :],
                    _bcast_mid(stbl[:, sc, 32:64], NHL))
                # t2 = qraw * cos (cos same for both halves)
                nc.vector.tensor_mul(
                    t2.rearrange("p (hh k) -> p hh k", k=32),
                    qraw.rearrange("p (hh k) -> p hh k", k=32),
                    _bcast_mid(ctbl[:, sc, :], 2 * NHL))
                qn = rop.tile([P, DH], BF16, tag="qn")
                nc.vector.tensor_add(qn, t1, t2)
                # transpose all 4 [128,128] blocks in one xbar DMA:
                # out[p, j, f] = in[f, j*128 + p]
                nc.sync.dma_start_transpose(
                    out=dst_store[:, :, sc * P:(sc + 1) * P], in_=qn)

            def proj_q(sc):
                sl = slice(sc * P, (sc + 1) * P)
                pq = ps_pp.tile([P, DH], F32, tag="pp", name="ppt")
                proj_mm(pq, xh, xl, wqh, wql, sl)
                rope_and_transpose(pq, sc, qt_store, 1.0 / (8.0 * GAMMA))

            def proj_k(sc):
                sl = slice(sc * P, (sc + 1) * P)
                pk = ps_pp.tile([P, DH], F32, tag="pp", name="ppt")
                proj_mm(pk, xh, xl, wkh, wkl, sl)
                rope_and_transpose(pk, sc, kt_store, 1.0 / GAMMA)

            def proj_v(sc):
                sl = slice(sc * P, (sc + 1) * P)
                pv = ps_pp.tile([P, DH], F32, tag="pp", name="ppt")
                proj_mm(pv, xh, xl, wvh, wvl, sl)
                nc.vector.tensor_scalar_mul(
                    v_aug[:, sc, :, 0:64],
                    pv.rearrange("p (h c) -> p h c", c=64), 1.0 / GAMMA)

            def proj_chunk(sc):
                proj_q(sc)
                proj_k(sc)
                proj_v(sc)

            def proj_term(ps, xa, wa, sl, ti):
                for t in range(4):
                    nc.tensor.matmul(
                        ps, xa[:, 2 * t:2 * t + 2, sl],
                        wa[:, 2 * t:2 * t + 2, :],
                        start=(ti == 0 and t == 0),
                        stop=(ti == 2 and t == 3),
                        perf_mode=DR)

            def proj_parts(sc, which):
                # split one projection into 3 filler-sized pieces
                sl = slice(sc * P, (sc + 1) * P)
                wh, wl = {"q": (wqh, wql), "k": (wkh, wkl),
                          "v": (wvh, wvl)}[which]
                cell = {}

                def p1():
                    cell["ps"] = ps_pp.tile([P, DH], F32, tag="pp",
                                            name="ppt")
                    proj_term(cell["ps"], xh, wh, sl, 0)

                def p2():
                    proj_term(cell["ps"], xh, wl, sl, 1)

                def p3():
                    proj_term(cell["ps"], xl, wh, sl, 2)
                    if which == "q":
                        rope_and_transpose(cell["ps"], sc, qt_store,
                                           1.0 / (8.0 * GAMMA))
                    elif which == "k":
                        rope_and_transpose(cell["ps"], sc, kt_store,
                                           1.0 / GAMMA)
                    else:
                        nc.vector.tensor_scalar_mul(
                            v_aug[:, sc, :, 0:64],
                            cell["ps"].rearrange("p (h c) -> p h c", c=64),
                            1.0 / GAMMA)
                return [p1, p2, p3]

            # ---- attention for one q-block (generator: yields between
            # score-pair stages so filler work can be interleaved) ----
            def attn_block(qb, ott, on_pair=None):
                nk = 4 * qb + 4            # valid key tiles for this q-block
                q0 = qb * 512
                xdef = []      # cross-pair deferred work (block-3 outproj)
                for pair in range(4):
                    otn = otn_pool.tile([P, 4, 2, 64], BF16, tag="otn")
                    carried = []   # head A's deferred PV tail + normalize
                    for hloc in range(2):
                        h = 2 * pair + hloc
                        hp = slice(64 * hloc, 64 * hloc + 64)
                        pv_ps = ps_pv.tile([P, 4, 65], F32, tag="pv")
                        state = {"first": True}

                        def mk_pv(pt, kp, pv_ps=pv_ps, h=h, state=state):
                            def go():
                                for i in range(2):
                                    kt = 2 * kp + i
                                    tmin = max(0, kt - 4 * qb)
                                    for t in range(tmin, 4):
                                        last = (kt == 4 * qb + t)
                                        nc.tensor.matmul(
                                            pv_ps[:, t, :],
                                            pt[:, i, 128 * t:128 * t + 128],
                                            v_aug[:, kt, h, :],
                                            start=state["first"],
                                            stop=(last and t == 3),
                                            skip_group_check=True)
                                        state["first"] = False
                            return go

                        def mk_tail(pv_ps=pv_ps, hl=hloc):
                            def go():
                                rl = rlp.tile([P, 4, 1], F32, tag="rl",
                                              name="rl")
                                nc.vector.reciprocal(rl, pv_ps[:, :, 64:65])
                                nc.vector.tensor_mul(
                                    otn[:, :, hl, :], pv_ps[:, :, 0:64],
                                    _bcast_last(rl, 64))
                            return go

                        pv_queue = []
                        for kp in range((nk + 1) // 2):
                            sps = ps_sc.tile([P, 2, 512], F32, tag="sc")
                            pt = ptp.tile([P, 2, 512], BF16, tag="pt")
                            for i in range(2):
                                kt = 2 * kp + i
                                diag = kt - 4 * qb  # >= 0 on diagonal tiles
                                lo = max(0, 128 * diag)
                                nc.tensor.matmul(
                                    sps[:, i, lo:512],
                                    kt_store[hp, pair, kt * P:(kt + 1) * P],
                                    qt_store[hp, pair, q0 + lo:q0 + 512],
                                    start=True, stop=True)
                            lo0 = max(0, 128 * (2 * kp - 4 * qb))
                            nc.scalar.activation(
                                out=pt.rearrange("p a b -> p (a b)")[:, lo0:1024],
                                in_=sps.rearrange("p a b -> p (a b)")[:, lo0:1024],
                                func=AF.Exp)
                            for i in range(2):
                                kt = 2 * kp + i
                                diag = kt - 4 * qb
                                if diag >= 0:
                                    # zero the causally-dead upper-tri block
                                    lo = 128 * diag
                                    nc.vector.tensor_mul(
                                        pt[:, i, lo:lo + 128],
                                        pt[:, i, lo:lo + 128], ntri)
                            pv_queue.append(mk_pv(pt, kp))
                            if len(pv_queue) > 2:
                                pv_queue.pop(0)()
                            if carried:
                                carried.pop(0)()
                            if xdef:
                                xdef.pop(0)()
                            yield
                        if hloc == 0:
                            # defer head A's PV tail + normalize into head
                            # B's pair stream
                            carried = pv_queue + [mk_tail()]
                        else:
                            for f in pv_queue:
                                f()
                            for f in carried:
                                f()
                            mk_tail()()
                    # transpose otn [q, t, d2] -> ott [d2, t, q] in one xbar DMA
                    nc.sync.dma_start_transpose(
                        out=ott.rearrange("p a (t b) -> p a t b", t=4)[:, pair, :, :],
                        in_=otn.rearrange("p t h k -> p t (h k)"))
                    if on_pair is not None:
                        xdef += on_pair(pair)
                for f in xdef:
                    f()

            def outproj_ec(qb, ott, ec, tail=False):
                q0 = qb * 512
                yps = ps_pp.tile([P, DH], F32, tag="pp", name="ppt")
                for p_i in range(4):
                    nc.tensor.matmul(
                        yps, wot[:, p_i, ec * P:(ec + 1) * P], ott[:, p_i, :],
                        start=(p_i == 0), stop=(p_i == 3))
                ysb = ysb_pool.tile([P, DH], F32, tag="ysb")
                nc.vector.tensor_copy(ysb, yps)
                nc.sync.dma_start(
                    out=yt_d[ec * P:(ec + 1) * P, q0:q0 + 512],
                    in_=ysb)

            # ---- software-pipelined emission: proj chunks and outproj of
            # the previous block are spread evenly through the attention
            # pair stream (fillers consumed at score-pair yields) ----
            n_yields = {qb: 8 * ((4 * qb + 4 + 1) // 2) for qb in range(QB)}
            for _rep in range(repeat):
                for sc in range(4):
                    proj_chunk(sc)
                prev = None    # (qb, ott) awaiting output projection
                deferred = []  # late k/v proj items pushed into the next block
                for p_blk in range(QB):
                    early = deferred
                    deferred = []
                    fillers = []
                    if prev is not None:
                        qb_prev, ott_prev = prev
                        fillers += [
                            (lambda e=ec, q=qb_prev, o=ott_prev:
                             outproj_ec(q, o, e)) for ec in range(8)]
                    if p_blk + 1 < QB:
                        chunks = list(range(4 * p_blk + 4, 4 * p_blk + 8))
                        for sc in chunks:
                            fillers.append(lambda s=sc: proj_q(s))
                        for sc in chunks[:2]:
                            fillers += [lambda s=sc: proj_k(s),
                                        lambda s=sc: proj_v(s)]
                        kv_late = [(lambda s=sc: proj_k(s), lambda s=sc: proj_v(s))
                                   for sc in chunks[2:]]
                        if p_blk + 1 >= 2:
                            for fk, fv in kv_late:
                                deferred += [fk, fv]
                        else:
                            for fk, fv in kv_late:
                                fillers += [fk, fv]
                    # round-robin interleave outproj and proj fillers
                    opj = [f for i, f in enumerate(fillers) if i < 8 and prev is not None]
                    prj = fillers[len(opj):]
                    mixed = []
                    while opj or prj:
                        if prj:
                            mixed.append(prj.pop(0))
                        if opj:
                            mixed.append(opj.pop(0))
                    fillers = mixed
                    # deferred items run at the earliest yields; the rest
                    # spread across the block
                    ny = n_yields[p_blk]
                    sched = {}
                    for fi, f in enumerate(early):
                        sched.setdefault(1 + fi, []).append(f)
                    nf = len(fillers)
                    for fi in range(nf):
                        sched.setdefault(1 + len(early) + (fi * (ny - len(early)))
                                         // max(nf, 1), []).append(fillers[fi])
                    ott = ott_pool.tile([P, 4, 512], BF16, tag="ott")
                    on_pair = None
                    if p_blk == QB - 1:
                        yac = [yac_pool.tile([P, DH], F32, tag="yac",
                                             name=f"yac{e}") for e in range(8)]

                        def on_pair(pair, yac=yac, q0=p_blk * 512, ott=ott):
                            def mk(ec, pair=pair):
                                def go():
                                    yps = ps_pp.tile([P, DH], F32, tag="pp",
                                                     name="ypp")
                                    nc.tensor.matmul(
                                        yps, wot[:, pair, ec * P:(ec + 1) * P],
                                        ott[:, pair, :], start=True, stop=True)
                                    if pair == 0:
                                        nc.vector.tensor_copy(yac[ec], yps)
                                    else:
                                        nc.vector.tensor_add(
                                            yac[ec], yac[ec], yps)
                                    if pair == 3:
                                        nc.sync.dma_start(
                                            out=yt_d[ec * P:(ec + 1) * P,
                                                     q0:q0 + 512],
                                            in_=yac[ec])
                                return go
                            return [mk(ec) for ec in range(8)]
                    for yi, _ in enumerate(attn_block(p_blk, ott, on_pair)):
                        for f in sched.pop(yi, []):
                            f()
                    for rest in sorted(sched):
                        for f in sched[rest]:
                            f()
                    prev = (p_blk, ott)

    nc.compile()
    return nc


_NC = None


def _get_program():
    global _NC
    if _NC is None:
        _NC = build_program()
    return _NC


def _prep_inputs(x, token_positions, Wq, Wk, Wv, Wo):
    B = x.shape[0]
    bf = ml_dtypes.bfloat16
    f8 = ml_dtypes.float8_e4m3
    # rope tables from token_positions
    pos = np.asarray(token_positions, dtype=np.float64)
    k = np.arange(1, 33, dtype=np.float64)
    denom = np.power(THETA, 2.0 * (k - 1.0) / 64.0)
    ang = pos[:, None] / denom[None, :]              # [S, 32]
    cos_t = np.cos(ang).astype(np.float32)
    sin_t = np.sin(ang).astype(np.float32)
    ct = cos_t.reshape(SC, P, 32).transpose(1, 0, 2).astype(bf)    # [128, 16, 32]
    st = np.concatenate([-sin_t, sin_t], axis=1)                   # [S, 64]
    st = st.reshape(SC, P, 64).transpose(1, 0, 2).astype(bf)       # [128, 16, 64]

    # deinterleave permutation within each head (evens then odds)
    permh = np.concatenate([np.arange(0, 64, 2), np.arange(1, 64, 2)])
    perm = (np.arange(16)[:, None] * 64 + permh[None, :]).reshape(-1)  # [1024]

    # lower-inclusive 0/1 causal mask [128, 128] (keys x q)
    a = np.arange(P)
    ntri = np.where(a[:, None] <= a[None, :], 1.0, 0.0).astype(bf)

    def hilo_T(W):
        # W [rows=512, 1024] -> hi/lo fp8 in [128, KT, 512] transposed layout
        hi = W.astype(f8)
        lo = (W - hi.astype(np.float32)).astype(f8)
        out = []
        for m in (hi, lo):
            mt = m.T.reshape(KT, P, DH).transpose(1, 0, 2)
            out.append(np.ascontiguousarray(mt))
        return out

    Wq_s = np.asarray(Wq, np.float32) * GAMMA
    Wk_s = np.asarray(Wk, np.float32) * GAMMA
    Wv_s = np.asarray(Wv, np.float32) * GAMMA
    Wo_s = np.asarray(Wo, np.float32)

    in_maps = []
    for b in range(B):
        xT = np.ascontiguousarray(np.asarray(x[b], np.float32).T)   # [1024, 2048]
        xT_h = xT.astype(f8)
        xT_l = (xT - xT_h.astype(np.float32)).astype(f8)
        xh_h = np.ascontiguousarray(xT_h.reshape(KT, P, S).transpose(1, 0, 2))
        xl_h = np.ascontiguousarray(xT_l.reshape(KT, P, S).transpose(1, 0, 2))
        for g in range(2):
            rows = slice(g * DH, (g + 1) * DH)
            wqh_h, wql_h = hilo_T(Wq_s[perm[rows], :])
            wkh_h, wkl_h = hilo_T(Wk_s[perm[rows], :])
            wvh_h, wvl_h = hilo_T(Wv_s[rows, :])
            wo_g = Wo_s[:, g * DH:(g + 1) * DH]          # [1024, 512]
            wot_h = wo_g.T.reshape(4, P, DM).transpose(1, 0, 2).astype(bf)
            in_maps.append({
                "xh": xh_h,
                "xl": xl_h,
                "wqh": wqh_h, "wql": wql_h,
                "wkh": wkh_h, "wkl": wkl_h,
                "wvh": wvh_h, "wvl": wvl_h,
                "wot": np.ascontiguousarray(wot_h),
                "ct": np.ascontiguousarray(ct),
                "st": np.ascontiguousarray(st),
                "ntri": np.ascontiguousarray(ntri),
            })
    return in_maps


def kernel(x, token_positions, Wq, Wk, Wv, Wo, _trace=False):
    nc = _get_program()
    in_maps = _prep_inputs(x, token_positions, Wq, Wk, Wv, Wo)
    res = run_bass_kernel_spmd(nc, in_maps, list(range(8)), trace=_trace)
    B = x.shape[0]
    out = np.zeros((B, S, DM), np.float32)
    for b in range(B):
        for g in range(2):
            out[b] += res.results[2 * b + g]["yt"].T
    if _trace:
        return out, res
    return out


# revision 61
# speedup vs baseline: 1.0017x; 1.0017x over previous
"""Causal MHA with RoPE on 8 trn2 cores.

Sharding: core = (batch b, head-group g). b = core//2, g = core%2.
Each core computes 8 heads of one batch and a partial output projection;
host sums the two partials per batch.

Per-core pipeline:
  - Q/K/V projections as fp8(e4m3) hi/lo DoubleRow matmuls (3-term, host
    quantized with gamma=64 pre-scale; dequant folded into the psum->sbuf
    scaled copy on DVE).
  - RoPE on DVE (bf16), transposes via single 3D xbar DMAs.
  - scores [keys x q] bf16 with ragged causal extents, exp on ACT,
    post-exp 0/1 tri-mask on DVE for diagonal tiles, PV oriented
    [q x (64+1)] with causal tile skipping; softmax normalize via DVE
    reciprocal + broadcast mul.
  - output projection bf16; proj chunks and the previous block's output
    projection are spread through the attention pair stream as fillers.
"""

import numpy as np
import ml_dtypes

import concourse.bass as bass
from concourse import bacc
import concourse.mybir as mybir
import concourse.tile as tile
from concourse.bass_utils import run_bass_kernel_spmd

F32 = mybir.dt.float32
BF16 = mybir.dt.bfloat16
F8 = mybir.dt.float8e4
AF = mybir.ActivationFunctionType
DR = mybir.MatmulPerfMode.DoubleRow

P = 128
S = 2048          # sequence length
DM = 1024         # model dim
DH = 512          # per-core heads dim (8 heads x 64)
DK = 64
NHL = 8           # local heads
KT = DM // P      # 8 k-tiles for projections
SC = S // P       # 16 s-chunks
QB = S // 512     # 4 q-blocks
THETA = 10000.0
GAMMA = 64.0      # fp8 weight pre-scale


def _bcast_last(ap3, n):
    # [P, A, 1] -> [P, A, n] with stride-0 last dim
    assert len(ap3.ap) == 3 and ap3.ap[2][1] == 1
    return bass.AP(tensor=ap3.tensor, offset=ap3.offset,
                   ap=[ap3.ap[0], ap3.ap[1], [0, n]])


def _bcast_mid(ap2d, n):
    # [P, K] -> [P, n, K] with a stride-0 middle dim
    assert len(ap2d.ap) == 2
    return bass.AP(tensor=ap2d.tensor, offset=ap2d.offset,
                   ap=[ap2d.ap[0], [0, n], ap2d.ap[1]])


def build_program(repeat=1):
    nc = bacc.Bacc("TRN2", target_bir_lowering=False, debug=False)
    xh_d = nc.dram_tensor("xh", [P, KT, S], F8, kind="ExternalInput").ap()
    xl_d = nc.dram_tensor("xl", [P, KT, S], F8, kind="ExternalInput").ap()
    wqh_d = nc.dram_tensor("wqh", [P, KT, DH], F8, kind="ExternalInput").ap()
    wql_d = nc.dram_tensor("wql", [P, KT, DH], F8, kind="ExternalInput").ap()
    wkh_d = nc.dram_tensor("wkh", [P, KT, DH], F8, kind="ExternalInput").ap()
    wkl_d = nc.dram_tensor("wkl", [P, KT, DH], F8, kind="ExternalInput").ap()
    wvh_d = nc.dram_tensor("wvh", [P, KT, DH], F8, kind="ExternalInput").ap()
    wvl_d = nc.dram_tensor("wvl", [P, KT, DH], F8, kind="ExternalInput").ap()
    wot_d = nc.dram_tensor("wot", [P, 4, DM], BF16, kind="ExternalInput").ap()
    ct_d = nc.dram_tensor("ct", [P, SC, 32], BF16, kind="ExternalInput").ap()
    st_d = nc.dram_tensor("st", [P, SC, 64], BF16, kind="ExternalInput").ap()
    ntri_d = nc.dram_tensor("ntri", [P, P], BF16, kind="ExternalInput").ap()
    yt_d = nc.dram_tensor("yt", [DM, S], F32, kind="ExternalOutput").ap()

    with tile.TileContext(nc) as tc:
        with (
            tc.tile_pool(name="consts", bufs=1) as consts,
            tc.tile_pool(name="raw", bufs=6) as raw,
            tc.tile_pool(name="rop", bufs=8) as rop,
            tc.tile_pool(name="ptp", bufs=6) as ptp,
            tc.tile_pool(name="otn", bufs=6) as otn_pool,
            tc.tile_pool(name="ott", bufs=4) as ott_pool,
            tc.tile_pool(name="rlp", bufs=6) as rlp,
            tc.tile_pool(name="ysb", bufs=4) as ysb_pool,
            tc.tile_pool(name="yac", bufs=8) as yac_pool,
            tc.tile_pool(name="pssc", bufs=2, space="PSUM") as ps_sc,
            tc.tile_pool(name="pspv", bufs=2, space="PSUM") as ps_pv,
            tc.tile_pool(name="pspp", bufs=2, space="PSUM") as ps_pp,
        ):
            # ---- constants (ordered so the first proj chunk starts early) ----
            xh = consts.tile([P, KT, S], F8)
            xl = consts.tile([P, KT, S], F8)
            wqh = consts.tile([P, KT, DH], F8)
            nc.sync.dma_start(out=wqh, in_=wqh_d)
            nc.sync.dma_start(out=xh[:, :, 0:512], in_=xh_d[:, :, 0:512])
            wql = consts.tile([P, KT, DH], F8)
            nc.sync.dma_start(out=wql, in_=wql_d)
            nc.sync.dma_start(out=xl[:, :, 0:512], in_=xl_d[:, :, 0:512])
            wkh = consts.tile([P, KT, DH], F8)
            nc.sync.dma_start(out=wkh, in_=wkh_d)
            wkl = consts.tile([P, KT, DH], F8)
            nc.sync.dma_start(out=wkl, in_=wkl_d)
            wvh = consts.tile([P, KT, DH], F8)
            nc.sync.dma_start(out=wvh, in_=wvh_d)
            wvl = consts.tile([P, KT, DH], F8)
            nc.sync.dma_start(out=wvl, in_=wvl_d)
            ctbl = consts.tile([P, SC, 32], BF16)
            nc.sync.dma_start(out=ctbl, in_=ct_d)
            stbl = consts.tile([P, SC, 64], BF16)
            nc.sync.dma_start(out=stbl, in_=st_d)
            for sb in range(1, 4):
                nc.sync.dma_start(out=xh[:, :, sb * 512:(sb + 1) * 512],
                                  in_=xh_d[:, :, sb * 512:(sb + 1) * 512])
                nc.sync.dma_start(out=xl[:, :, sb * 512:(sb + 1) * 512],
                                  in_=xl_d[:, :, sb * 512:(sb + 1) * 512])
            ntri = consts.tile([P, P], BF16)
            nc.sync.dma_start(out=ntri, in_=ntri_d)
            wot = consts.tile([P, 4, DM], BF16)
            nc.sync.dma_start(out=wot, in_=wot_d)

            qt_store = consts.tile([P, 4, S], BF16)
            kt_store = consts.tile([P, 4, S], BF16)
            v_aug = consts.tile([P, SC, NHL, 65], BF16)
            nc.vector.memset(v_aug[:, :, :, 64:65], 1.0)

            # pre-zero the two score psum slots: ragged diagonal scores leave
            # the causally-dead prefix unwritten, but exp still reads it
            for _zi in range(2):
                zs = ps_sc.tile([P, 2, 512], F32, tag="sc")
                nc.vector.memset(zs, 0.0)

            # ---- projections + rope, per s-chunk ----
            def proj_mm(ps, xhi, xlo, whi, wlo, sl):
                # 3-term fp8 hi/lo DoubleRow: 12 matmuls, 256-contraction each
                terms = [(xhi, whi), (xhi, wlo), (xlo, whi)]
                for ti, (xa, wa) in enumerate(terms):
                    for t in range(4):
                        nc.tensor.matmul(
                            ps, xa[:, 2 * t:2 * t + 2, sl], wa[:, 2 * t:2 * t + 2, :],
                            start=(ti == 0 and t == 0),
                            stop=(ti == 2 and t == 3),
                            perf_mode=DR)

            def rope_and_transpose(pj, sc, dst_store, deq):
                qraw = raw.tile([P, DH], BF16, tag="qraw")
                nc.vector.tensor_scalar_mul(qraw, pj, deq)
                t1 = rop.tile([P, DH], BF16, tag="t1")
                t2 = rop.tile([P, DH], BF16, tag="t2")
                qv = qraw.rearrange("p (h two k) -> p h two k", two=2, k=32)
                t1v = t1.rearrange("p (h two k) -> p h two k", two=2, k=32)
                # evens-out: t1[:, h, 0, :] = qraw[:, h, 1, :] * (-sin)
                nc.vector.tensor_mul(
                    t1v[:, :, 0, :], qv[:, :, 1, :],
                    _bcast_mid(stbl[:, sc, 0:32], NHL))
                # odds-out: t1[:, h, 1, :] = qraw[:, h, 0, :] * (+sin)
                nc.vector.tensor_mul(
                    t1v[:, :, 1, :], qv[:, :, 0, :],
                    _bcast_mid(stbl[:, sc, 32:64], NHL))
                # t2 = qraw * cos (cos same for both halves)
                nc.vector.tensor_mul(
                    t2.rearrange("p (hh k) -> p hh k", k=32),
                    qraw.rearrange("p (hh k) -> p hh k", k=32),
                    _bcast_mid(ctbl[:, sc, :], 2 * NHL))
                qn = rop.tile([P, DH], BF16, tag="qn")
                nc.vector.tensor_add(qn, t1, t2)
                # transpose all 4 [128,128] blocks in one xbar DMA:
                # out[p, j, f] = in[f, j*128 + p]
                nc.sync.dma_start_transpose(
                    out=dst_store[:, :, sc * P:(sc + 1) * P], in_=qn)

            def proj_q(sc):
                sl = slice(sc * P, (sc + 1) * P)
                pq = ps_pp.tile([P, DH], F32, tag="pp", name="ppt")
                proj_mm(pq, xh, xl, wqh, wql, sl)
                rope_and_transpose(pq, sc, qt_store, 1.0 / (8.0 * GAMMA))

            def proj_k(sc):
                sl = slice(sc * P, (sc + 1) * P)
                pk = ps_pp.tile([P, DH], F32, tag="pp", name="ppt")
                proj_mm(pk, xh, xl, wkh, wkl, sl)
                rope_and_transpose(pk, sc, kt_store, 1.0 / GAMMA)

            def proj_v(sc):
                sl = slice(sc * P, (sc + 1) * P)
                pv = ps_pp.tile([P, DH], F32, tag="pp", name="ppt")
                proj_mm(pv, xh, xl, wvh, wvl, sl)
                nc.vector.tensor_scalar_mul(
                    v_aug[:, sc, :, 0:64],
                    pv.rearrange("p (h c) -> p h c", c=64), 1.0 / GAMMA)

            def proj_chunk(sc):
                proj_q(sc)
                proj_k(sc)
                proj_v(sc)

            def proj_term(ps, xa, wa, sl, ti):
                for t in range(4):
                    nc.tensor.matmul(
                        ps, xa[:, 2 * t:2 * t + 2, sl],
                        wa[:, 2 * t:2 * t + 2, :],
                        start=(ti == 0 and t == 0),
                        stop=(ti == 2 and t == 3),
                        perf_mode=DR)

            def proj_parts(sc, which):
                # split one projection into 3 filler-sized pieces
                sl = slice(sc * P, (sc + 1) * P)
                wh, wl = {"q": (wqh, wql), "k": (wkh, wkl),
                          "v": (wvh, wvl)}[which]
                cell = {}

                def p1():
                    cell["ps"] = ps_pp.tile([P, DH], F32, tag="pp",
                                            name="ppt")
                    proj_term(cell["ps"], xh, wh, sl, 0)

                def p2():
                    proj_term(cell["ps"], xh, wl, sl, 1)

                def p3():
                    proj_term(cell["ps"], xl, wh, sl, 2)
                    if which == "q":
                        rope_and_transpose(cell["ps"], sc, qt_store,
                                           1.0 / (8.0 * GAMMA))
                    elif which == "k":
                        rope_and_transpose(cell["ps"], sc, kt_store,
                                           1.0 / GAMMA)
                    else:
                        nc.vector.tensor_scalar_mul(
                            v_aug[:, sc, :, 0:64],
                            cell["ps"].rearrange("p (h c) -> p h c", c=64),
                            1.0 / GAMMA)
                return [p1, p2, p3]

            # ---- attention for one q-block (generator: yields between
            # score-pair stages so filler work can be interleaved) ----
            def attn_block(qb, ott, on_pair=None):
                nk = 4 * qb + 4            # valid key tiles for this q-block
                q0 = qb * 512
                xdef = []      # cross-pair deferred work (block-3 outproj)
                for pair in range(4):
                    otn = otn_pool.tile([P, 4, 2, 64], BF16, tag="otn")
                    carried = []   # head A's deferred PV tail + normalize
                    for hloc in range(2):
                        h = 2 * pair + hloc
                        hp = slice(64 * hloc, 64 * hloc + 64)
                        pv_ps = ps_pv.tile([P, 4, 65], F32, tag="pv")
                        state = {"first": True}

                        def mk_pv(pt, kp, pv_ps=pv_ps, h=h, state=state):
                            def go():
                                for i in range(2):
                                    kt = 2 * kp + i
                                    tmin = max(0, kt - 4 * qb)
                                    for t in range(tmin, 4):
                                        last = (kt == 4 * qb + t)
                                        nc.tensor.matmul(
                                            pv_ps[:, t, :],
                                            pt[:, i, 128 * t:128 * t + 128],
                                            v_aug[:, kt, h, :],
                                            start=state["first"],
                                            stop=(last and t == 3),
                                            skip_group_check=True)
                                        state["first"] = False
                            return go

                        def mk_tail(pv_ps=pv_ps, hl=hloc):
                            def go():
                                rl = rlp.tile([P, 4, 1], F32, tag="rl",
                                              name="rl")
                                nc.vector.reciprocal(rl, pv_ps[:, :, 64:65])
                                nc.vector.tensor_mul(
                                    otn[:, :, hl, :], pv_ps[:, :, 0:64],
                                    _bcast_last(rl, 64))
                            return go

                        pv_queue = []
                        for kp in range((nk + 1) // 2):
                            sps = ps_sc.tile([P, 2, 512], F32, tag="sc")
                            pt = ptp.tile([P, 2, 512], BF16, tag="pt")
                            for i in range(2):
                                kt = 2 * kp + i
                                diag = kt - 4 * qb  # >= 0 on diagonal tiles
                                lo = max(0, 128 * diag)
                                nc.tensor.matmul(
                                    sps[:, i, lo:512],
                                    kt_store[hp, pair, kt * P:(kt + 1) * P],
                                    qt_store[hp, pair, q0 + lo:q0 + 512],
                                    start=True, stop=True)
                            lo0 = max(0, 128 * (2 * kp - 4 * qb))
                            nc.scalar.activation(
                                out=pt.rearrange("p a b -> p (a b)")[:, lo0:1024],
                                in_=sps.rearrange("p a b -> p (a b)")[:, lo0:1024],
                                func=AF.Exp)
                            for i in range(2):
                                kt = 2 * kp + i
                                diag = kt - 4 * qb
                                if diag >= 0:
                                    # zero the causally-dead upper-tri block
                                    lo = 128 * diag
                                    nc.vector.tensor_mul(
                                        pt[:, i, lo:lo + 128],
                                        pt[:, i, lo:lo + 128], ntri)
                            pv_queue.append(mk_pv(pt, kp))
                            if len(pv_queue) > 3:
                                pv_queue.pop(0)()
                            if carried:
                                carried.pop(0)()
                            if xdef:
                                xdef.pop(0)()
                            yield
                        if hloc == 0:
                            # defer head A's PV tail + normalize into head
                            # B's pair stream
                            carried = pv_queue + [mk_tail()]
                        else:
                            for f in pv_queue:
                                f()
                            for f in carried:
                                f()
                            mk_tail()()
                    # transpose otn [q, t, d2] -> ott [d2, t, q] in one xbar DMA
                    nc.sync.dma_start_transpose(
                        out=ott.rearrange("p a (t b) -> p a t b", t=4)[:, pair, :, :],
                        in_=otn.rearrange("p t h k -> p t (h k)"))
                    if on_pair is not None:
                        xdef += on_pair(pair)
                for f in xdef:
                    f()

            def outproj_ec(qb, ott, ec, tail=False):
                q0 = qb * 512
                yps = ps_pp.tile([P, DH], F32, tag="pp", name="ppt")
                for p_i in range(4):
                    nc.tensor.matmul(
                        yps, wot[:, p_i, ec * P:(ec + 1) * P], ott[:, p_i, :],
                        start=(p_i == 0), stop=(p_i == 3))
                ysb = ysb_pool.tile([P, DH], F32, tag="ysb")
                nc.vector.tensor_copy(ysb, yps)
                nc.sync.dma_start(
                    out=yt_d[ec * P:(ec + 1) * P, q0:q0 + 512],
                    in_=ysb)

            # ---- software-pipelined emission: proj chunks and outproj of
            # the previous block are spread evenly through the attention
            # pair stream (fillers consumed at score-pair yields) ----
            n_yields = {qb: 8 * ((4 * qb + 4 + 1) // 2) for qb in range(QB)}
            for _rep in range(repeat):
                for sc in range(4):
                    proj_chunk(sc)
                prev = None    # (qb, ott) awaiting output projection
                deferred = []  # late k/v proj items pushed into the next block
                for p_blk in range(QB):
                    early = deferred
                    deferred = []
                    fillers = []
                    if prev is not None:
                        qb_prev, ott_prev = prev
                        fillers += [
                            (lambda e=ec, q=qb_prev, o=ott_prev:
                             outproj_ec(q, o, e)) for ec in range(8)]
                    if p_blk + 1 < QB:
                        chunks = list(range(4 * p_blk + 4, 4 * p_blk + 8))
                        for sc in chunks:
                            fillers.append(lambda s=sc: proj_q(s))
                        for sc in chunks[:2]:
                            fillers += [lambda s=sc: proj_k(s),
                                        lambda s=sc: proj_v(s)]
                        kv_late = [(lambda s=sc: proj_k(s), lambda s=sc: proj_v(s))
                                   for sc in chunks[2:]]
                        if p_blk + 1 >= 2:
                            for fk, fv in kv_late:
                                deferred += [fk, fv]
                        else:
                            for fk, fv in kv_late:
                                fillers += [fk, fv]
                    # round-robin interleave outproj and proj fillers
                    opj = [f for i, f in enumerate(fillers) if i < 8 and prev is not None]
                    prj = fillers[len(opj):]
                    mixed = []
                    while opj or prj:
                        if prj:
                            mixed.append(prj.pop(0))
                        if opj:
                            mixed.append(opj.pop(0))
                    fillers = mixed
                    # deferred items run at the earliest yields; the rest
                    # spread across the block
                    ny = n_yields[p_blk]
                    sched = {}
                    for fi, f in enumerate(early):
                        sched.setdefault(1 + fi, []).append(f)
                    nf = len(fillers)
                    for fi in range(nf):
                        sched.setdefault(1 + len(early) + (fi * (ny - len(early)))
                                         // max(nf, 1), []).append(fillers[fi])
                    ott = ott_pool.tile([P, 4, 512], BF16, tag="ott")
                    on_pair = None
                    if p_blk == QB - 1:
                        yac = [yac_pool.tile([P, DH], F32, tag="yac",
                                             name=f"yac{e}") for e in range(8)]

                        def on_pair(pair, yac=yac, q0=p_blk * 512, ott=ott):
                            def mk(ec, pair=pair):
                                def go():
                                    yps = ps_pp.tile([P, DH], F32, tag="pp",
                                                     name="ypp")
                                    nc.tensor.matmul(
                                        yps, wot[:, pair, ec * P:(ec + 1) * P],
                                        ott[:, pair, :], start=True, stop=True)
                                    if pair == 0:
                                        nc.vector.tensor_copy(yac[ec], yps)
                                    else:
                                        nc.vector.tensor_add(
                                            yac[ec], yac[ec], yps)
                                    if pair == 3:
                                        nc.sync.dma_start(
                                            out=yt_d[ec * P:(ec + 1) * P,
                                                     q0:q0 + 512],
                                            in_=yac[ec])
                                return go
                            return [mk(ec) for ec in range(8)]
                    for yi, _ in enumerate(attn_block(p_blk, ott, on_pair)):
                        for f in sched.pop(yi, []):
                            f()
                    for rest in sorted(sched):
                        for f in sched[rest]:
                            f()
                    prev = (p_blk, ott)

    nc.compile()
    return nc


_NC = None


def _get_program():
    global _NC
    if _NC is None:
        _NC = build_program()
    return _NC


def _prep_inputs(x, token_positions, Wq, Wk, Wv, Wo):
    B = x.shape[0]
    bf = ml_dtypes.bfloat16
    f8 = ml_dtypes.float8_e4m3
    # rope tables from token_positions
    pos = np.asarray(token_positions, dtype=np.float64)
    k = np.arange(1, 33, dtype=np.float64)
    denom = np.power(THETA, 2.0 * (k - 1.0) / 64.0)
    ang = pos[:, None] / denom[None, :]              # [S, 32]
    cos_t = np.cos(ang).astype(np.float32)
    sin_t = np.sin(ang).astype(np.float32)
    ct = cos_t.reshape(SC, P, 32).transpose(1, 0, 2).astype(bf)    # [128, 16, 32]
    st = np.concatenate([-sin_t, sin_t], axis=1)                   # [S, 64]
    st = st.reshape(SC, P, 64).transpose(1, 0, 2).astype(bf)       # [128, 16, 64]

    # deinterleave permutation within each head (evens then odds)
    permh = np.concatenate([np.arange(0, 64, 2), np.arange(1, 64, 2)])
    perm = (np.arange(16)[:, None] * 64 + permh[None, :]).reshape(-1)  # [1024]

    # lower-inclusive 0/1 causal mask [128, 128] (keys x q)
    a = np.arange(P)
    ntri = np.where(a[:, None] <= a[None, :], 1.0, 0.0).astype(bf)

    def hilo_T(W):
        # W [rows=512, 1024] -> hi/lo fp8 in [128, KT, 512] transposed layout
        hi = W.astype(f8)
        lo = (W - hi.astype(np.float32)).astype(f8)
        out = []
        for m in (hi, lo):
            mt = m.T.reshape(KT, P, DH).transpose(1, 0, 2)
            out.append(np.ascontiguousarray(mt))
        return out

    Wq_s = np.asarray(Wq, np.float32) * GAMMA
    Wk_s = np.asarray(Wk, np.float32) * GAMMA
    Wv_s = np.asarray(Wv, np.float32) * GAMMA
    Wo_s = np.asarray(Wo, np.float32)

    in_maps = []
    for b in range(B):
        xT = np.ascontiguousarray(np.asarray(x[b], np.float32).T)   # [1024, 2048]
        xT_h = xT.astype(f8)
        xT_l = (xT - xT_h.astype(np.float32)).astype(f8)
        xh_h = np.ascontiguousarray(xT_h.reshape(KT, P, S).transpose(1, 0, 2))
        xl_h = np.ascontiguousarray(xT_l.reshape(KT, P, S).transpose(1, 0, 2))
        for g in range(2):
            rows = slice(g * DH, (g + 1) * DH)
            wqh_h, wql_h = hilo_T(Wq_s[perm[rows], :])
            wkh_h, wkl_h = hilo_T(Wk_s[perm[rows], :])
            wvh_h, wvl_h = hilo_T(Wv_s[rows, :])
            wo_g = Wo_s[:, g * DH:(g + 1) * DH]          # [1024, 512]
            wot_h = wo_g.T.reshape(4, P, DM).transpose(1, 0, 2).astype(bf)
            in_maps.append({
                "xh": xh_h,
                "xl": xl_h,
                "wqh": wqh_h, "wql": wql_h,
                "wkh": wkh_h, "wkl": wkl_h,
                "wvh": wvh_h, "wvl": wvl_h,
                "wot": np.ascontiguousarray(wot_h),
                "ct": np.ascontiguousarray(ct),
                "st": np.ascontiguousarray(st),
                "ntri": np.ascontiguousarray(ntri),
            })
    return in_maps


def kernel(x, token_positions, Wq, Wk, Wv, Wo, _trace=False):
    nc = _get_program()
    in_maps = _prep_inputs(x, token_positions, Wq, Wk, Wv, Wo)
    res = run_bass_kernel_spmd(nc, in_maps, list(range(8)), trace=_trace)
    B = x.shape[0]
    out = np.zeros((B, S, DM), np.float32)
    for b in range(B):
        for g in range(2):
            out[b] += res.results[2 * b + g]["yt"].T
    if _trace:
        return out, res
    return out
